# revision 14
# baseline (speedup 1.0000x reference)
"""Trainium2 Bass kernel for batched KNN-interpolation MSE (nn_KnnMSE).

Problem: B=16 graphs; per graph, for each of N2=2048 query points find the
K=3 nearest of N1=2048 source points (by 3-D coords), inverse-square-distance
interpolate F=64 source features, and return MSE against the query features.

Sharding: data-parallel over B across 8 NeuronCores (2 graphs/core).

Wall-clock on the axon tunnel is dominated by wire bytes (~200 MB/s, ~70 ms
RTT), so inputs are shipped quantized — coords as f16, features as fp8e4m3
(measured rel-err of the final MSE vs f32 inputs: ~5e-4, tolerance 2e-2) —
packed into a SINGLE u8 operand (one wire buffer is faster and much more
jitter-robust than 4), bitcast + upcast to f32 on-chip. The SPMD executable
is built and jit-compiled once and cached across calls.

Per graph on-core:
  - upcast coord/feature tiles to f32.
  - PE computes g[q,n] = 2*c2.c1 - |c1|^2 (= |c2|^2 - d2) via K=4 matmuls
    with the c1 norm folded into the contraction (aug row).
  - DVE max8/max_index extract the top-3 (largest g = smallest d2) values and
    indices per query row.
  - weights w = 1/max(d2,1e-16) with d2 = |c2|^2 - g  (tiny [128,3] ops).
  - one hardware dma_gather per k fetches neighbor feature rows (256B each)
    from a packed f32 DRAM copy of f1.
  - fused scalar_tensor_tensor ops do the weighted sum, normalize, subtract
    f2 and accumulate per-partition sums of squared errors.
Each core reduces its SSE to a [128, 1] column; the host sums the 8 cores'
partials in float64.
"""

import numpy as np
import ml_dtypes

import concourse.bass as bass
import concourse.tile as tile
import concourse.masks as masks
from concourse import bacc, mybir
from concourse import bass_utils

F32 = mybir.dt.float32
F16 = mybir.dt.float16
F8 = mybir.dt.float8e4
U32 = mybir.dt.uint32
U8 = mybir.dt.uint8
ALU = mybir.AluOpType
AX = mybir.AxisListType

B, N, F, K = 16, 2048, 64, 3
CORES = 8
NB = B // CORES          # graphs per core = 2
P = 128                  # partitions
T = N // P               # q-tiles per graph = 16


def build_program():
    nc = bacc.Bacc(
        "TRN2",
        target_bir_lowering=False,
        debug=False,
        enable_asserts=False,
        num_devices=CORES,
    )

    # One packed u8 operand: row r = [txc_r f16x3 | txf_r fp8x64 | pxc_r | pxf_r]
    # (a single wire buffer is measurably faster and far more jitter-robust on
    # the axon tunnel than 4 separate operands)
    RB = 2 * (2 * 3 + F)  # 140 bytes per row
    pk = nc.dram_tensor("pk", [NB * N, RB], U8, kind="ExternalInput")
    out = nc.dram_tensor("out", [P, 1], F32, kind="ExternalOutput")

    with tile.TileContext(nc) as tc:
        from contextlib import ExitStack

        with ExitStack() as ctx:
            const_pool = ctx.enter_context(tc.tile_pool(name="const", bufs=1))
            in_pool = ctx.enter_context(tc.tile_pool(name="inp", bufs=2))
            mat_pool = ctx.enter_context(tc.tile_pool(name="mat", bufs=2))
            g_pool = ctx.enter_context(tc.tile_pool(name="gs", bufs=4))
            topk_pool = ctx.enter_context(tc.tile_pool(name="topk", bufs=2))
            small_pool = ctx.enter_context(tc.tile_pool(name="small", bufs=6))
            psum_pool = ctx.enter_context(
                tc.tile_pool(name="ps", bufs=8, space="PSUM")
            )
            dram_pool = ctx.enter_context(
                tc.tile_pool(name="dram", bufs=2, space="DRAM")
            )

            ident = const_pool.tile([P, P], F32, tag="ident")
            masks.make_identity(nc, ident[:])
            sse_all = const_pool.tile([P, NB * T], F32, tag="sse")

            for b in range(NB):
                rows = slice(b * N, (b + 1) * N)

                # ---- load packed bytes, bitcast slices, upcast to f32
                pkt = in_pool.tile([P, T, RB], U8, tag="pkt")
                nc.sync.dma_start(
                    pkt[:], pk[rows, :].rearrange("(t p) c -> p t c", p=P)
                )

                c1t = in_pool.tile([P, T, 3], F32, tag="c1t")
                nc.vector.tensor_copy(c1t[:], pkt[:, :, 0:6].bitcast(F16))
                f1t = in_pool.tile([P, T, F], F32, tag="f1t")
                nc.vector.tensor_copy(f1t[:], pkt[:, :, 6 : 6 + F].bitcast(F8))
                c2t = in_pool.tile([P, T, 3], F32, tag="c2t")
                nc.vector.tensor_copy(
                    c2t[:], pkt[:, :, 6 + F : 12 + F].bitcast(F16)
                )
                f2t_all = in_pool.tile([P, T, F], F32, tag="f2t")
                nc.vector.tensor_copy(
                    f2t_all[:], pkt[:, :, 12 + F : RB].bitcast(F8)
                )

                # ---- packed f32 f1 copy in DRAM (gather source, 256B rows)
                f1pk = dram_pool.tile([N, F], F32, tag="f1pk")
                nc.sync.dma_start(
                    f1pk[:].rearrange("(t p) c -> p t c", p=P), f1t[:]
                )

                # ---- build matmul operand matrices
                # tmp1[p,t,0:3] = 2*c1 ; tmp1[p,t,3] = -|c1|^2
                tmp1 = mat_pool.tile([P, T, 4], F32, tag="tmp1")
                sq3 = mat_pool.tile([P, T, 3], F32, tag="sq3")
                nc.vector.tensor_mul(sq3[:], c1t[:], c1t[:])
                nc.vector.tensor_reduce(
                    tmp1[:, :, 3:4], sq3[:], axis=AX.X, op=ALU.add
                )
                nc.vector.tensor_scalar_mul(tmp1[:, :, 3:4], tmp1[:, :, 3:4], -1.0)
                nc.vector.tensor_scalar_mul(tmp1[:, :, 0:3], c1t[:], 2.0)

                # tmp2[p,t,0:3] = c2 ; tmp2[p,t,3] = 1
                tmp2 = mat_pool.tile([P, T, 4], F32, tag="tmp2")
                nc.scalar.copy(tmp2[:, :, 0:3], c2t[:])
                nc.gpsimd.memset(tmp2[:, :, 3:4], 1.0)

                # |c2|^2 per query, natural layout [128, 16]
                c2n = mat_pool.tile([P, T], F32, tag="c2n")
                sq4 = mat_pool.tile([P, T, 3], F32, tag="sq4")
                nc.vector.tensor_mul(sq4[:], c2t[:], c2t[:])
                nc.vector.tensor_reduce(c2n[:], sq4[:], axis=AX.X, op=ALU.add)

                # transpose tmp1/tmp2 -> r1a [4, 2048] (rhs), c2a [4, 2048] (lhsT)
                r1a = mat_pool.tile([4, N], F32, tag="r1a")
                c2a = mat_pool.tile([4, N], F32, tag="c2a")
                for h in range(4):
                    ptr1 = psum_pool.tile([P, 512], F32, tag="ps")
                    for u in range(4):
                        t = h * 4 + u
                        nc.tensor.transpose(
                            ptr1[0:4, u * P : (u + 1) * P], tmp1[:, t, :], ident[:]
                        )
                    nc.scalar.copy(r1a[:, h * 512 : (h + 1) * 512], ptr1[0:4, :])
                    ptr2 = psum_pool.tile([P, 512], F32, tag="ps")
                    for u in range(4):
                        t = h * 4 + u
                        nc.tensor.transpose(
                            ptr2[0:4, u * P : (u + 1) * P], tmp2[:, t, :], ident[:]
                        )
                    nc.scalar.copy(c2a[:, h * 512 : (h + 1) * 512], ptr2[0:4, :])

                # ---- phase 1: distances + top-3 per q-tile
                dca = topk_pool.tile([P, T * K], F32, tag="dca")   # clipped d2 of top3
                nbrall = topk_pool.tile([P, T, K, F], F32, tag="nbrall")
                for t in range(T):
                    gs = g_pool.tile([P, N], F32, tag="gs")
                    for j in range(4):
                        pg = psum_pool.tile([P, 512], F32, tag="ps")
                        nc.tensor.matmul(
                            pg[:],
                            c2a[:, t * P : (t + 1) * P],
                            r1a[:, j * 512 : (j + 1) * 512],
                            start=True,
                            stop=True,
                        )
                        nc.scalar.copy(gs[:, j * 512 : (j + 1) * 512], pg[:])

                    m8 = small_pool.tile([P, 8], F32, tag="m8")
                    i8 = small_pool.tile([P, 8], U32, tag="i8")
                    nc.vector.max(m8[:], gs[:])
                    nc.vector.max_index(i8[:], m8[:], gs[:])

                    # d2_top3 = |c2|^2 - g_top3, clipped at 1e-16
                    dslice = dca[:, K * t : K * t + K]
                    nc.vector.tensor_scalar(
                        dslice,
                        m8[:, 0:K],
                        -1.0,
                        c2n[:, t : t + 1],
                        op0=ALU.mult,
                        op1=ALU.add,
                    )
                    nc.vector.tensor_scalar_max(dslice, dslice, 1e-16)

                    for k in range(K):
                        nc.gpsimd.indirect_dma_start(
                            out=nbrall[:, t, k, :],
                            out_offset=None,
                            in_=f1pk[:],
                            in_offset=bass.IndirectOffsetOnAxis(
                                ap=i8[:, k : k + 1], axis=0
                            ),
                        )

                # ---- weights for all tiles at once
                wca = topk_pool.tile([P, T * K], F32, tag="wca")
                dena = topk_pool.tile([P, T], F32, tag="dena")
                rdena = topk_pool.tile([P, T], F32, tag="rdena")
                nc.vector.reciprocal(wca[:], dca[:])
                nc.vector.tensor_reduce(
                    dena[:],
                    wca[:].rearrange("p (t k) -> p t k", k=K),
                    axis=AX.X,
                    op=ALU.add,
                )
                nc.vector.reciprocal(rdena[:], dena[:])

                # ---- interpolation + squared error per q-tile
                for t in range(T):
                    f2t = f2t_all[:, t, :]
                    acc = small_pool.tile([P, F], F32, tag="acc")
                    nc.scalar.activation(
                        acc[:],
                        nbrall[:, t, 0, :],
                        mybir.ActivationFunctionType.Copy,
                        scale=wca[:, K * t : K * t + 1],
                    )
                    nc.vector.scalar_tensor_tensor(
                        acc[:],
                        nbrall[:, t, 1, :],
                        wca[:, K * t + 1 : K * t + 2],
                        acc[:],
                        op0=ALU.mult,
                        op1=ALU.add,
                    )
                    nc.vector.scalar_tensor_tensor(
                        acc[:],
                        nbrall[:, t, 2, :],
                        wca[:, K * t + 2 : K * t + 3],
                        acc[:],
                        op0=ALU.mult,
                        op1=ALU.add,
                    )
                    diff = small_pool.tile([P, F], F32, tag="diff")
                    nc.vector.scalar_tensor_tensor(
                        diff[:],
                        acc[:],
                        rdena[:, t : t + 1],
                        f2t,
                        op0=ALU.mult,
                        op1=ALU.subtract,
                    )
                    junk = small_pool.tile([P, F], F32, tag="junk")
                    nc.scalar.activation(
                        junk[:],
                        diff[:],
                        mybir.ActivationFunctionType.Square,
                        accum_out=sse_all[:, b * T + t : b * T + t + 1],
                    )

            sse_tot = const_pool.tile([P, 1], F32, tag="sse_tot")
            nc.vector.tensor_reduce(sse_tot[:], sse_all[:], axis=AX.X, op=ALU.add)
            nc.sync.dma_start(out[:], sse_tot[:])

    nc.compile()
    return nc


_NC = None


def _get_nc():
    global _NC
    if _NC is None:
        _NC = build_program()
    return _NC


_QUANT_JIT = None


def _quantize_np(true_x, pred_x):
    return np.concatenate(
        [
            true_x[:, :3].astype(np.float16).view(np.uint8),
            true_x[:, 3:].astype(ml_dtypes.float8_e4m3).view(np.uint8),
            pred_x[:, :3].astype(np.float16).view(np.uint8),
            pred_x[:, 3:].astype(ml_dtypes.float8_e4m3).view(np.uint8),
        ],
        axis=1,
    )


def _quantize(true_x, pred_x):
    """Pack f16 coords + fp8e4m3 features into one u8 wire buffer per row.
    XLA:CPU casts are ~7x faster than ml_dtypes' numpy loop, so jit the
    cast+pack on the host CPU when possible."""
    global _QUANT_JIT
    true_x = np.asarray(true_x, dtype=np.float32)
    pred_x = np.asarray(pred_x, dtype=np.float32)
    if _QUANT_JIT is None:
        try:
            import jax
            import jax.numpy as jnp

            cpu = jax.devices("cpu")[0]

            def _q(tx, px):
                n = tx.shape[0]
                return jnp.concatenate(
                    [
                        jax.lax.bitcast_convert_type(
                            tx[:, :3].astype(jnp.float16), jnp.uint8
                        ).reshape(n, 6),
                        jax.lax.bitcast_convert_type(
                            tx[:, 3:].astype(jnp.float8_e4m3), jnp.uint8
                        ).reshape(n, F),
                        jax.lax.bitcast_convert_type(
                            px[:, :3].astype(jnp.float16), jnp.uint8
                        ).reshape(n, 6),
                        jax.lax.bitcast_convert_type(
                            px[:, 3:].astype(jnp.float8_e4m3), jnp.uint8
                        ).reshape(n, F),
                    ],
                    axis=1,
                )

            jit_q = jax.jit(_q, device=cpu)
            z = np.zeros((2, 3 + F), np.float32)
            ref = jit_q(z, z)
            assert np.array_equal(np.asarray(ref), _quantize_np(z, z))
            _QUANT_JIT = jit_q
        except Exception:
            _QUANT_JIT = False
    if _QUANT_JIT:
        return np.asarray(_QUANT_JIT(true_x, pred_x))
    return _quantize_np(true_x, pred_x)


# ---------------------------------------------------------------------------
# Cached SPMD runner (axon / PJRT path).
#
# bass_utils.run_bass_kernel_spmd rebuilds and retraces a fresh
# jax.jit(shard_map(...)) on every call (~150 ms of host work per call).
# This runner builds the identical jitted executable once and reuses it;
# the per-call cost is then just operand transfer + execution + fetch.
# ---------------------------------------------------------------------------

_RUNNER = None


def _build_runner(nc):
    import jax
    from jax.sharding import Mesh, PartitionSpec
    from jax.experimental.shard_map import shard_map
    from concourse.bass2jax import (
        _bass_exec_p,
        install_neuronx_cc_hook,
        partition_id_tensor,
    )

    install_neuronx_cc_hook()

    partition_name = nc.partition_id_tensor.name if nc.partition_id_tensor else None
    in_names, out_names, out_avals = [], [], []
    for alloc in nc.m.functions[0].allocations:
        if not isinstance(alloc, mybir.MemoryLocationSet):
            continue
        name = alloc.memorylocations[0].name
        if alloc.kind == "ExternalInput":
            if name != partition_name:
                in_names.append(name)
        elif alloc.kind == "ExternalOutput":
            out_names.append(name)
            out_avals.append(
                jax.core.ShapedArray(tuple(alloc.tensor_shape), mybir.dt.np(alloc.dtype))
            )
    n_params = len(in_names)
    n_outs = len(out_avals)
    all_in_names = list(in_names) + list(out_names)
    if partition_name is not None:
        all_in_names.append(partition_name)

    def _body(*args):
        operands = list(args)
        if partition_name is not None:
            operands.append(partition_id_tensor())
        return tuple(
            _bass_exec_p.bind(
                *operands,
                out_avals=tuple(out_avals),
                in_names=tuple(all_in_names),
                out_names=tuple(out_names),
                lowering_input_output_aliases=(),
                sim_require_finite=True,
                sim_require_nnan=True,
                nc=nc,
            )
        )

    devices = [d for d in jax.devices() if d.platform != "cpu"][:CORES]
    if len(devices) < CORES:
        devices = jax.devices()[:CORES]
    assert len(devices) == CORES, f"need {CORES} devices, have {len(jax.devices())}"
    mesh = Mesh(np.asarray(devices), ("core",))
    in_specs = (PartitionSpec("core"),) * (n_params + n_outs)
    out_specs = (PartitionSpec("core"),) * n_outs
    donate = tuple(range(n_params, n_params + n_outs))
    sharded = jax.jit(
        shard_map(_body, mesh=mesh, in_specs=in_specs, out_specs=out_specs,
                  check_rep=False),
        donate_argnums=donate,
        keep_unused=True,
    )
    zero_shapes = [
        ((CORES * a.shape[0], *a.shape[1:]), a.dtype) for a in out_avals
    ]
    assert in_names == ["pk"], in_names

    def run(packed):
        zeros = [np.zeros(s, d) for s, d in zero_shapes]
        outs = sharded(packed, *zeros)
        # np.asarray immediately (no block first): the fetch RPC pipelines
        # behind the execute on the tunnel stream.
        return [np.asarray(o) for o in outs]

    return run


def _get_runner():
    global _RUNNER
    if _RUNNER is None:
        _RUNNER = _build_runner(_get_nc())
    return _RUNNER


def kernel(true_x, pred_x, batch1=None, batch2=None, **_):
    packed = _quantize(true_x, pred_x)
    if bass_utils.axon_active():
        (out,) = _get_runner()(packed)
        total = out.astype(np.float64).sum()
    else:
        nc = _get_nc()
        in_maps = []
        for c in range(CORES):
            sl = slice(c * NB * N, (c + 1) * NB * N)
            in_maps.append({"pk": np.ascontiguousarray(packed[sl])})
        res = bass_utils.run_bass_kernel_spmd(nc, in_maps, core_ids=list(range(CORES)))
        total = sum(r["out"].astype(np.float64).sum() for r in res.results)
    return np.float32(total / (B * N * F))


# revision 15
# speedup vs baseline: 1.6320x; 1.6320x over previous
"""Trainium2 Bass kernel for batched KNN-interpolation MSE (nn_KnnMSE).

Problem: B=16 graphs; per graph, for each of N2=2048 query points find the
K=3 nearest of N1=2048 source points (by 3-D coords), inverse-square-distance
interpolate F=64 source features, and return MSE against the query features.

Sharding: data-parallel over B across 8 NeuronCores (2 graphs/core).

Wall-clock on the axon tunnel is dominated by wire bytes (~200 MB/s, ~70 ms
RTT), so inputs are shipped quantized — coords as f16, features as fp8e4m3
(measured rel-err of the final MSE vs f32 inputs: ~5e-4, tolerance 2e-2) —
packed into a SINGLE u8 operand (one wire buffer is faster and much more
jitter-robust than 4), bitcast + upcast to f32 on-chip. The SPMD executable
is built and jit-compiled once and cached across calls.

Per graph on-core:
  - upcast coord/feature tiles to f32.
  - PE computes g[q,n] = 2*c2.c1 - |c1|^2 (= |c2|^2 - d2) via K=4 matmuls
    with the c1 norm folded into the contraction (aug row).
  - DVE max8/max_index extract the top-3 (largest g = smallest d2) values and
    indices per query row.
  - weights w = 1/max(d2,1e-16) with d2 = |c2|^2 - g  (tiny [128,3] ops).
  - one hardware dma_gather per k fetches neighbor feature rows (256B each)
    from a packed f32 DRAM copy of f1.
  - fused scalar_tensor_tensor ops do the weighted sum, normalize, subtract
    f2 and accumulate per-partition sums of squared errors.
Each core reduces its SSE to a [128, 1] column; the host sums the 8 cores'
partials in float64.
"""

import numpy as np
import ml_dtypes

import concourse.bass as bass
import concourse.tile as tile
import concourse.masks as masks
from concourse import bacc, mybir
from concourse import bass_utils

F32 = mybir.dt.float32
F16 = mybir.dt.float16
F8 = mybir.dt.float8e4
U32 = mybir.dt.uint32
U8 = mybir.dt.uint8
ALU = mybir.AluOpType
AX = mybir.AxisListType

B, N, F, K = 16, 2048, 64, 3
CORES = 8
NB = B // CORES          # graphs per core = 2
P = 128                  # partitions
T = N // P               # q-tiles per graph = 16


def build_program():
    nc = bacc.Bacc(
        "TRN2",
        target_bir_lowering=False,
        debug=False,
        enable_asserts=False,
        num_devices=CORES,
    )

    # One packed u8 operand: row r = [txc_r f16x3 | txf_r fp8x64 | pxc_r | pxf_r]
    # (a single wire buffer is measurably faster and far more jitter-robust on
    # the axon tunnel than 4 separate operands)
    RB = 2 * (2 * 3 + F)  # 140 bytes per row
    pk = nc.dram_tensor("pk", [NB * N, RB], U8, kind="ExternalInput")
    out = nc.dram_tensor("out", [P, 1], F32, kind="ExternalOutput")

    with tile.TileContext(nc) as tc:
        from contextlib import ExitStack

        with ExitStack() as ctx:
            const_pool = ctx.enter_context(tc.tile_pool(name="const", bufs=1))
            in_pool = ctx.enter_context(tc.tile_pool(name="inp", bufs=2))
            mat_pool = ctx.enter_context(tc.tile_pool(name="mat", bufs=2))
            g_pool = ctx.enter_context(tc.tile_pool(name="gs", bufs=4))
            topk_pool = ctx.enter_context(tc.tile_pool(name="topk", bufs=2))
            small_pool = ctx.enter_context(tc.tile_pool(name="small", bufs=6))
            psum_pool = ctx.enter_context(
                tc.tile_pool(name="ps", bufs=8, space="PSUM")
            )
            dram_pool = ctx.enter_context(
                tc.tile_pool(name="dram", bufs=2, space="DRAM")
            )

            ident = const_pool.tile([P, P], F32, tag="ident")
            masks.make_identity(nc, ident[:])
            sse_all = const_pool.tile([P, NB * T], F32, tag="sse")

            for b in range(NB):
                rows = slice(b * N, (b + 1) * N)

                # ---- load packed bytes, bitcast slices, upcast to f32
                pkt = in_pool.tile([P, T, RB], U8, tag="pkt")
                nc.sync.dma_start(
                    pkt[:], pk[rows, :].rearrange("(t p) c -> p t c", p=P)
                )

                c1t = in_pool.tile([P, T, 3], F32, tag="c1t")
                nc.vector.tensor_copy(c1t[:], pkt[:, :, 0:6].bitcast(F16))
                f1t = in_pool.tile([P, T, F], F32, tag="f1t")
                nc.vector.tensor_copy(f1t[:], pkt[:, :, 6 : 6 + F].bitcast(F8))
                c2t = in_pool.tile([P, T, 3], F32, tag="c2t")
                nc.vector.tensor_copy(
                    c2t[:], pkt[:, :, 6 + F : 12 + F].bitcast(F16)
                )
                f2t_all = in_pool.tile([P, T, F], F32, tag="f2t")
                nc.vector.tensor_copy(
                    f2t_all[:], pkt[:, :, 12 + F : RB].bitcast(F8)
                )

                # ---- packed f32 f1 copy in DRAM (gather source, 256B rows)
                f1pk = dram_pool.tile([N, F], F32, tag="f1pk")
                nc.sync.dma_start(
                    f1pk[:].rearrange("(t p) c -> p t c", p=P), f1t[:]
                )

                # ---- build matmul operand matrices
                # tmp1[p,t,0:3] = 2*c1 ; tmp1[p,t,3] = -|c1|^2
                tmp1 = mat_pool.tile([P, T, 4], F32, tag="tmp1")
                sq3 = mat_pool.tile([P, T, 3], F32, tag="sq3")
                nc.vector.tensor_mul(sq3[:], c1t[:], c1t[:])
                nc.vector.tensor_reduce(
                    tmp1[:, :, 3:4], sq3[:], axis=AX.X, op=ALU.add
                )
                nc.vector.tensor_scalar_mul(tmp1[:, :, 3:4], tmp1[:, :, 3:4], -1.0)
                nc.vector.tensor_scalar_mul(tmp1[:, :, 0:3], c1t[:], 2.0)

                # tmp2[p,t,0:3] = c2 ; tmp2[p,t,3] = 1
                tmp2 = mat_pool.tile([P, T, 4], F32, tag="tmp2")
                nc.scalar.copy(tmp2[:, :, 0:3], c2t[:])
                nc.gpsimd.memset(tmp2[:, :, 3:4], 1.0)

                # |c2|^2 per query, natural layout [128, 16]
                c2n = mat_pool.tile([P, T], F32, tag="c2n")
                sq4 = mat_pool.tile([P, T, 3], F32, tag="sq4")
                nc.vector.tensor_mul(sq4[:], c2t[:], c2t[:])
                nc.vector.tensor_reduce(c2n[:], sq4[:], axis=AX.X, op=ALU.add)

                # transpose tmp1/tmp2 -> r1a [4, 2048] (rhs), c2a [4, 2048] (lhsT)
                r1a = mat_pool.tile([4, N], F32, tag="r1a")
                c2a = mat_pool.tile([4, N], F32, tag="c2a")
                for h in range(4):
                    ptr1 = psum_pool.tile([P, 512], F32, tag="ps")
                    for u in range(4):
                        t = h * 4 + u
                        nc.tensor.transpose(
                            ptr1[0:4, u * P : (u + 1) * P], tmp1[:, t, :], ident[:]
                        )
                    nc.scalar.copy(r1a[:, h * 512 : (h + 1) * 512], ptr1[0:4, :])
                    ptr2 = psum_pool.tile([P, 512], F32, tag="ps")
                    for u in range(4):
                        t = h * 4 + u
                        nc.tensor.transpose(
                            ptr2[0:4, u * P : (u + 1) * P], tmp2[:, t, :], ident[:]
                        )
                    nc.scalar.copy(c2a[:, h * 512 : (h + 1) * 512], ptr2[0:4, :])

                # ---- phase 1: distances + top-3 per q-tile
                dca = topk_pool.tile([P, T * K], F32, tag="dca")   # clipped d2 of top3
                nbrall = topk_pool.tile([P, T, K, F], F32, tag="nbrall")
                for t in range(T):
                    gs = g_pool.tile([P, N], F32, tag="gs")
                    for j in range(4):
                        pg = psum_pool.tile([P, 512], F32, tag="ps")
                        nc.tensor.matmul(
                            pg[:],
                            c2a[:, t * P : (t + 1) * P],
                            r1a[:, j * 512 : (j + 1) * 512],
                            start=True,
                            stop=True,
                        )
                        nc.scalar.copy(gs[:, j * 512 : (j + 1) * 512], pg[:])

                    m8 = small_pool.tile([P, 8], F32, tag="m8")
                    i8 = small_pool.tile([P, 8], U32, tag="i8")
                    nc.vector.max(m8[:], gs[:])
                    nc.vector.max_index(i8[:], m8[:], gs[:])

                    # d2_top3 = |c2|^2 - g_top3, clipped at 1e-16
                    dslice = dca[:, K * t : K * t + K]
                    nc.vector.tensor_scalar(
                        dslice,
                        m8[:, 0:K],
                        -1.0,
                        c2n[:, t : t + 1],
                        op0=ALU.mult,
                        op1=ALU.add,
                    )
                    nc.vector.tensor_scalar_max(dslice, dslice, 1e-16)

                    for k in range(K):
                        nc.gpsimd.indirect_dma_start(
                            out=nbrall[:, t, k, :],
                            out_offset=None,
                            in_=f1pk[:],
                            in_offset=bass.IndirectOffsetOnAxis(
                                ap=i8[:, k : k + 1], axis=0
                            ),
                        )

                # ---- weights for all tiles at once
                wca = topk_pool.tile([P, T * K], F32, tag="wca")
                dena = topk_pool.tile([P, T], F32, tag="dena")
                rdena = topk_pool.tile([P, T], F32, tag="rdena")
                nc.vector.reciprocal(wca[:], dca[:])
                nc.vector.tensor_reduce(
                    dena[:],
                    wca[:].rearrange("p (t k) -> p t k", k=K),
                    axis=AX.X,
                    op=ALU.add,
                )
                nc.vector.reciprocal(rdena[:], dena[:])

                # ---- interpolation + squared error per q-tile
                for t in range(T):
                    f2t = f2t_all[:, t, :]
                    acc = small_pool.tile([P, F], F32, tag="acc")
                    nc.scalar.activation(
                        acc[:],
                        nbrall[:, t, 0, :],
                        mybir.ActivationFunctionType.Copy,
                        scale=wca[:, K * t : K * t + 1],
                    )
                    nc.vector.scalar_tensor_tensor(
                        acc[:],
                        nbrall[:, t, 1, :],
                        wca[:, K * t + 1 : K * t + 2],
                        acc[:],
                        op0=ALU.mult,
                        op1=ALU.add,
                    )
                    nc.vector.scalar_tensor_tensor(
                        acc[:],
                        nbrall[:, t, 2, :],
                        wca[:, K * t + 2 : K * t + 3],
                        acc[:],
                        op0=ALU.mult,
                        op1=ALU.add,
                    )
                    diff = small_pool.tile([P, F], F32, tag="diff")
                    nc.vector.scalar_tensor_tensor(
                        diff[:],
                        acc[:],
                        rdena[:, t : t + 1],
                        f2t,
                        op0=ALU.mult,
                        op1=ALU.subtract,
                    )
                    junk = small_pool.tile([P, F], F32, tag="junk")
                    nc.scalar.activation(
                        junk[:],
                        diff[:],
                        mybir.ActivationFunctionType.Square,
                        accum_out=sse_all[:, b * T + t : b * T + t + 1],
                    )

            sse_tot = const_pool.tile([P, 1], F32, tag="sse_tot")
            nc.vector.tensor_reduce(sse_tot[:], sse_all[:], axis=AX.X, op=ALU.add)
            nc.sync.dma_start(out[:], sse_tot[:])

    nc.compile()
    return nc


_NC = None


def _get_nc():
    global _NC
    if _NC is None:
        _NC = build_program()
    return _NC


_QUANT_JIT = None


def _quantize_np(true_x, pred_x):
    return np.concatenate(
        [
            true_x[:, :3].astype(np.float16).view(np.uint8),
            true_x[:, 3:].astype(ml_dtypes.float8_e4m3).view(np.uint8),
            pred_x[:, :3].astype(np.float16).view(np.uint8),
            pred_x[:, 3:].astype(ml_dtypes.float8_e4m3).view(np.uint8),
        ],
        axis=1,
    )


def _quantize(true_x, pred_x):
    """Pack f16 coords + fp8e4m3 features into one u8 wire buffer per row.
    XLA:CPU casts are ~7x faster than ml_dtypes' numpy loop, so jit the
    cast+pack on the host CPU when possible."""
    global _QUANT_JIT
    true_x = np.asarray(true_x, dtype=np.float32)
    pred_x = np.asarray(pred_x, dtype=np.float32)
    if _QUANT_JIT is None:
        try:
            import jax
            import jax.numpy as jnp

            cpu = jax.devices("cpu")[0]

            def _q(tx, px):
                n = tx.shape[0]
                return jnp.concatenate(
                    [
                        jax.lax.bitcast_convert_type(
                            tx[:, :3].astype(jnp.float16), jnp.uint8
                        ).reshape(n, 6),
                        jax.lax.bitcast_convert_type(
                            tx[:, 3:].astype(jnp.float8_e4m3), jnp.uint8
                        ).reshape(n, F),
                        jax.lax.bitcast_convert_type(
                            px[:, :3].astype(jnp.float16), jnp.uint8
                        ).reshape(n, 6),
                        jax.lax.bitcast_convert_type(
                            px[:, 3:].astype(jnp.float8_e4m3), jnp.uint8
                        ).reshape(n, F),
                    ],
                    axis=1,
                )

            jit_q = jax.jit(_q, device=cpu)
            z = np.zeros((2, 3 + F), np.float32)
            ref = jit_q(z, z)
            assert np.array_equal(np.asarray(ref), _quantize_np(z, z))
            _QUANT_JIT = jit_q
        except Exception:
            _QUANT_JIT = False
    if _QUANT_JIT:
        return np.asarray(_QUANT_JIT(true_x, pred_x))
    return _quantize_np(true_x, pred_x)


# ---------------------------------------------------------------------------
# Cached SPMD runner (axon / PJRT path).
#
# bass_utils.run_bass_kernel_spmd rebuilds and retraces a fresh
# jax.jit(shard_map(...)) on every call (~150 ms of host work per call).
# This runner builds the identical jitted executable once and reuses it;
# the per-call cost is then just operand transfer + execution + fetch.
# ---------------------------------------------------------------------------

_RUNNER = None


def _build_runner(nc):
    import jax
    from jax.sharding import Mesh, PartitionSpec
    from jax.experimental.shard_map import shard_map
    from concourse.bass2jax import (
        _bass_exec_p,
        install_neuronx_cc_hook,
        partition_id_tensor,
    )

    install_neuronx_cc_hook()

    partition_name = nc.partition_id_tensor.name if nc.partition_id_tensor else None
    in_names, out_names, out_avals = [], [], []
    for alloc in nc.m.functions[0].allocations:
        if not isinstance(alloc, mybir.MemoryLocationSet):
            continue
        name = alloc.memorylocations[0].name
        if alloc.kind == "ExternalInput":
            if name != partition_name:
                in_names.append(name)
        elif alloc.kind == "ExternalOutput":
            out_names.append(name)
            out_avals.append(
                jax.core.ShapedArray(tuple(alloc.tensor_shape), mybir.dt.np(alloc.dtype))
            )
    n_params = len(in_names)
    n_outs = len(out_avals)
    all_in_names = list(in_names) + list(out_names)
    if partition_name is not None:
        all_in_names.append(partition_name)

    def _body(*args):
        operands = list(args)
        if partition_name is not None:
            operands.append(partition_id_tensor())
        return tuple(
            _bass_exec_p.bind(
                *operands,
                out_avals=tuple(out_avals),
                in_names=tuple(all_in_names),
                out_names=tuple(out_names),
                lowering_input_output_aliases=(),
                sim_require_finite=True,
                sim_require_nnan=True,
                nc=nc,
            )
        )

    devices = [d for d in jax.devices() if d.platform != "cpu"][:CORES]
    if len(devices) < CORES:
        devices = jax.devices()[:CORES]
    assert len(devices) == CORES, f"need {CORES} devices, have {len(jax.devices())}"
    mesh = Mesh(np.asarray(devices), ("core",))
    in_specs = (PartitionSpec("core"),) * (n_params + n_outs)
    out_specs = (PartitionSpec("core"),) * n_outs
    # No donation: the NEFF fully writes every output element (verified
    # identical results), so the zero "output-init" operands are ballast.
    # Without donation they are never consumed and one device-resident
    # copy can be reused across calls — no per-call upload.
    sharded = jax.jit(
        shard_map(_body, mesh=mesh, in_specs=in_specs, out_specs=out_specs,
                  check_rep=False),
        keep_unused=True,
    )
    from jax.sharding import NamedSharding

    zsh = NamedSharding(mesh, PartitionSpec("core"))
    zeros_dev = [
        jax.device_put(
            np.zeros((CORES * a.shape[0], *a.shape[1:]), a.dtype), zsh
        )
        for a in out_avals
    ]
    jax.block_until_ready(zeros_dev)
    assert in_names == ["pk"], in_names

    def run(packed):
        outs = sharded(packed, *zeros_dev)
        # np.asarray immediately (no block first): the fetch RPC pipelines
        # behind the execute on the tunnel stream.
        return [np.asarray(o) for o in outs]

    return run


def _get_runner():
    global _RUNNER
    if _RUNNER is None:
        _RUNNER = _build_runner(_get_nc())
    return _RUNNER


def kernel(true_x, pred_x, batch1=None, batch2=None, **_):
    packed = _quantize(true_x, pred_x)
    if bass_utils.axon_active():
        (out,) = _get_runner()(packed)
        total = out.astype(np.float64).sum()
    else:
        nc = _get_nc()
        in_maps = []
        for c in range(CORES):
            sl = slice(c * NB * N, (c + 1) * NB * N)
            in_maps.append({"pk": np.ascontiguousarray(packed[sl])})
        res = bass_utils.run_bass_kernel_spmd(nc, in_maps, core_ids=list(range(CORES)))
        total = sum(r["out"].astype(np.float64).sum() for r in res.results)
    return np.float32(total / (B * N * F))
